# revision 2
# baseline (speedup 1.0000x reference)
"""Causal multi-head attention on 8 TRN2 NeuronCores, data-parallel over batch.

Per-core work (batch=1): q/k/v projections (f32r matmuls), per-head causal
softmax attention with bf16 probs/V, f32 output.

Host-side prep: inputs are transposed to [D_IN, L] so the on-device
projection matmuls (which contract over D_IN) need no on-device transpose.
"""

import sys

sys.path.insert(0, "/opt/trn_rl_repo")

import numpy as np

import concourse.bacc as bacc
import concourse.tile as tile
from concourse import mybir
from concourse.bass_utils import run_bass_kernel_spmd
from concourse.masks import make_causal_mask, make_identity

B, L, DIN, H, D = 8, 1024, 512, 8, 64
HD = H * D
F32 = mybir.dt.float32
F32R = mybir.dt.float32r
BF16 = mybir.dt.bfloat16
N_CORES = 8

_cached = {}


def _build():
    nc = bacc.Bacc("TRN2", target_bir_lowering=False, debug=False,
                   enable_asserts=False, num_devices=N_CORES)

    qt_d = nc.dram_tensor("qt", [DIN, L], F32R, kind="ExternalInput").ap()
    kt_d = nc.dram_tensor("kt", [DIN, L], F32R, kind="ExternalInput").ap()
    vt_d = nc.dram_tensor("vt", [DIN, L], F32R, kind="ExternalInput").ap()
    wq_d = nc.dram_tensor("wq", [DIN, HD], F32R, kind="ExternalInput").ap()
    wk_d = nc.dram_tensor("wk", [DIN, HD], F32R, kind="ExternalInput").ap()
    wv_d = nc.dram_tensor("wv", [DIN, HD], F32R, kind="ExternalInput").ap()
    out_d = nc.dram_tensor("out", [L, HD], F32, kind="ExternalOutput").ap()

    with tile.TileContext(nc) as tc:
        _body(tc, out_d, qt_d, kt_d, vt_d, wq_d, wk_d, wv_d)
    nc.compile()
    return nc


def _body(tc, out_d, qt_d, kt_d, vt_d, wq_d, wk_d, wv_d):
    nc = tc.nc
    from contextlib import ExitStack
    with ExitStack() as ctx:
        const = ctx.enter_context(tc.tile_pool(name="const", bufs=1))
        big = ctx.enter_context(tc.tile_pool(name="big", bufs=1))
        sb = ctx.enter_context(tc.tile_pool(name="sb", bufs=3))
        ps_big = ctx.enter_context(tc.tile_pool(name="psb", bufs=2, space="PSUM"))
        ps_sm = ctx.enter_context(tc.tile_pool(name="pss", bufs=2, space="PSUM"))

        ident = const.tile([128, 128], BF16)
        make_identity(nc, ident[:])
        cmask = const.tile([128, 128], F32)
        make_causal_mask(nc, cmask[:], mask_val=-1e9)

        # ---- load inputs (transposed activations + weights), chunked on DIN
        xq = big.tile([128, 4, L], F32R)
        xk = big.tile([128, 4, L], F32R)
        xv = big.tile([128, 4, L], F32R)
        wq = big.tile([128, 4, HD], F32R)
        wk = big.tile([128, 4, HD], F32R)
        wv = big.tile([128, 4, HD], F32R)
        for t, d in ((xq, qt_d), (xk, kt_d), (xv, vt_d)):
            nc.sync.dma_start(t[:], d.rearrange("(c p) l -> p c l", p=128))
        for t, d in ((wq, wq_d), (wk, wk_d), (wv, wv_d)):
            nc.sync.dma_start(t[:], d.rearrange("(c p) n -> p c n", p=128))

        # ---- projections
        # qT/kT in "transposed" form [hd, L]: psum = W_chunk.T @ XT_chunk
        qTs = big.tile([128, 4, L], F32R)
        kTs = big.tile([128, 4, L], F32R)
        v_sb = big.tile([128, 8, HD], BF16)
        for w_sb, dst, scale, eng in ((wq, qTs, 0.125, "act"), (wk, kTs, 1.0, "dve")):
            for t in range(4):
                for s in range(2):
                    pp = ps_big.tile([128, 512], F32, tag="S")
                    for c in range(4):
                        nc.tensor.matmul(
                            pp[:],
                            lhsT=w_sb[:, c, t * 128:(t + 1) * 128],
                            rhs=(xq if dst is qTs else xk)[:, c, s * 512:(s + 1) * 512],
                            start=(c == 0), stop=(c == 3))
                    if eng == "act":
                        nc.scalar.mul(dst[:, t, s * 512:(s + 1) * 512], pp[:], scale)
                    else:
                        nc.vector.tensor_copy(dst[:, t, s * 512:(s + 1) * 512], pp[:])
        # v in natural form [L, hd], bf16
        for lt in range(8):
            pp = ps_big.tile([128, 512], F32, tag="S")
            for c in range(4):
                nc.tensor.matmul(
                    pp[:],
                    lhsT=xv[:, c, lt * 128:(lt + 1) * 128],
                    rhs=wv[:, c, :],
                    start=(c == 0), stop=(c == 3))
            if lt % 2 == 0:
                nc.vector.tensor_copy(v_sb[:, lt, :], pp[:])
            else:
                nc.scalar.copy(v_sb[:, lt, :], pp[:])

        # ---- attention
        for qt in range(8):
            Lq0 = qt * 128
            sums = sb.tile([128, 8], F32, tag="sums")
            out_sb = sb.tile([128, HD], F32, tag="osb")
            for h in range(8):
                t, po = h // 2, (h % 2) * 64
                Lk = (qt + 1) * 128
                S = ps_big.tile([128, 1024], F32, tag="S")
                for w in range(0, Lk, 512):
                    n = min(512, Lk - w)
                    nc.tensor.matmul(
                        S[:, w:w + n],
                        lhsT=qTs[po:po + 64, t, Lq0:Lq0 + 128],
                        rhs=kTs[po:po + 64, t, w:w + n],
                        start=True, stop=True)
                # causal mask on the diagonal tile
                nc.vector.tensor_add(S[:, Lq0:Lq0 + 128], S[:, Lq0:Lq0 + 128], cmask[:])
                nm = sb.tile([128, 1], F32, tag="nm")
                nc.vector.reduce_max(nm[:], S[:, :Lk], axis=mybir.AxisListType.X,
                                     negate=True)
                pr = sb.tile([128, 1024], BF16, tag="pr")
                nc.scalar.activation(pr[:, :Lk], S[:, :Lk],
                                     mybir.ActivationFunctionType.Exp,
                                     bias=nm[:], scale=1.0,
                                     accum_out=sums[:, h:h + 1])
                av = ps_sm.tile([128, 64], F32, tag="av")
                for kc in range(qt + 1):
                    pT = ps_sm.tile([128, 128], BF16, tag="pT")
                    nc.tensor.transpose(pT[:], pr[:, kc * 128:(kc + 1) * 128], ident[:])
                    pTs = sb.tile([128, 128], BF16, tag="pTs")
                    if kc % 2 == 0:
                        nc.vector.tensor_copy(pTs[:], pT[:])
                    else:
                        nc.scalar.copy(pTs[:], pT[:])
                    nc.tensor.matmul(av[:], lhsT=pTs[:],
                                     rhs=v_sb[:, kc, h * 64:(h + 1) * 64],
                                     start=(kc == 0), stop=(kc == qt))
                if h % 2 == 0:
                    nc.vector.tensor_copy(out_sb[:, h * 64:(h + 1) * 64], av[:])
                else:
                    nc.scalar.copy(out_sb[:, h * 64:(h + 1) * 64], av[:])
            rec = sb.tile([128, 8], F32, tag="rec")
            nc.vector.reciprocal(rec[:], sums[:])
            for h in range(8):
                nc.vector.tensor_scalar_mul(out_sb[:, h * 64:(h + 1) * 64],
                                            out_sb[:, h * 64:(h + 1) * 64],
                                            rec[:, h:h + 1])
            nc.sync.dma_start(out_d[Lq0:Lq0 + 128, :], out_sb[:])


def kernel(Q_seq, K_seq, V_seq, WQ, WK, WV, _trace=False):
    if "nc" not in _cached:
        _cached["nc"] = _build()
    nc = _cached["nc"]

    in_maps = []
    for b in range(N_CORES):
        in_maps.append({
            "qt": np.ascontiguousarray(Q_seq[b].T).astype(np.float32),
            "kt": np.ascontiguousarray(K_seq[b].T).astype(np.float32),
            "vt": np.ascontiguousarray(V_seq[b].T).astype(np.float32),
            "wq": np.ascontiguousarray(WQ, dtype=np.float32),
            "wk": np.ascontiguousarray(WK, dtype=np.float32),
            "wv": np.ascontiguousarray(WV, dtype=np.float32),
        })
    res = run_bass_kernel_spmd(nc, in_maps, core_ids=list(range(N_CORES)),
                               trace=_trace)
    out = np.stack([res.results[b]["out"] for b in range(N_CORES)], axis=0)
    if _trace:
        kernel.last_exec_time_ns = res.exec_time_ns
        kernel.last_results = res
    return out


# revision 3
# speedup vs baseline: 1.5502x; 1.5502x over previous
"""Causal multi-head attention on 8 TRN2 NeuronCores, data-parallel over batch.

Per-core work (batch=1): q/k/v projections, per-head causal softmax
attention. All matmuls in fp16 (f32 PSUM accumulation); softmax max/exp in
f32. Host-side prep: inputs transposed to [D_IN, L] and cast to fp16 so the
on-device projections (contracting over D_IN) need no on-device transpose.
"""

import sys

sys.path.insert(0, "/opt/trn_rl_repo")

import numpy as np

import concourse.bacc as bacc
import concourse.tile as tile
from concourse import mybir
from concourse.bass_utils import run_bass_kernel_spmd
from concourse.masks import make_identity

B, L, DIN, H, D = 8, 1024, 512, 8, 64
HD = H * D
F32 = mybir.dt.float32
F16 = mybir.dt.float16
N_CORES = 8
MASK_VAL = -60000.0

_cached = {}


def _build():
    nc = bacc.Bacc("TRN2", target_bir_lowering=False, debug=False,
                   enable_asserts=False, num_devices=N_CORES)

    qt_d = nc.dram_tensor("qt", [DIN, L], F16, kind="ExternalInput").ap()
    kt_d = nc.dram_tensor("kt", [DIN, L], F16, kind="ExternalInput").ap()
    vt_d = nc.dram_tensor("vt", [DIN, L], F16, kind="ExternalInput").ap()
    wq_d = nc.dram_tensor("wq", [DIN, HD], F16, kind="ExternalInput").ap()
    wk_d = nc.dram_tensor("wk", [DIN, HD], F16, kind="ExternalInput").ap()
    wv_d = nc.dram_tensor("wv", [DIN, HD], F16, kind="ExternalInput").ap()
    out_d = nc.dram_tensor("out", [L, HD], F32, kind="ExternalOutput").ap()

    with tile.TileContext(nc) as tc:
        _body(tc, out_d, qt_d, kt_d, vt_d, wq_d, wk_d, wv_d)
    nc.compile()
    return nc


def _body(tc, out_d, qt_d, kt_d, vt_d, wq_d, wk_d, wv_d):
    nc = tc.nc
    from contextlib import ExitStack
    with ExitStack() as ctx:
        const = ctx.enter_context(tc.tile_pool(name="const", bufs=1))
        big = ctx.enter_context(tc.tile_pool(name="big", bufs=1))
        sb = ctx.enter_context(tc.tile_pool(name="sb", bufs=3))
        ps_big = ctx.enter_context(tc.tile_pool(name="psb", bufs=2, space="PSUM"))
        ps_sm = ctx.enter_context(tc.tile_pool(name="pss", bufs=2, space="PSUM"))

        ident = const.tile([128, 128], F16)
        make_identity(nc, ident[:])
        # cmaskT[k, q] = MASK_VAL where k > q else 0; S_diag += cmaskT.T @ I
        cmaskT = const.tile([128, 128], F16)
        nc.gpsimd.memset(cmaskT[:], MASK_VAL)
        nc.gpsimd.affine_select(
            out=cmaskT[:], in_=cmaskT[:],
            compare_op=mybir.AluOpType.is_gt, fill=0.0,
            base=0, pattern=[[-1, 128]], channel_multiplier=1)

        # ---- load inputs (transposed, fp16), one DMA per DIN-chunk piece
        xq = big.tile([128, 4, L], F16)
        xk = big.tile([128, 4, L], F16)
        xv = big.tile([128, 4, L], F16)
        wq = big.tile([128, 4, HD], F16)
        wk = big.tile([128, 4, HD], F16)
        wv = big.tile([128, 4, HD], F16)
        for t, d in ((wq, wq_d), (xq, qt_d), (wk, wk_d), (xk, kt_d),
                     (wv, wv_d), (xv, vt_d)):
            r = d.rearrange("(c p) l -> p c l", p=128)
            for c in range(4):
                nc.sync.dma_start(t[:, c, :], r[:, c, :])

        # ---- projections (fp16 matmuls, f32 psum)
        # qT/kT "transposed" form [hd, L]; v natural [L, hd]
        qTs = big.tile([128, 4, L], F16)
        kTs = big.tile([128, 4, L], F16)
        v_sb = big.tile([128, 8, HD], F16)
        for w_sb, x_sb, dst, scale in ((wq, xq, qTs, 0.125), (wk, xk, kTs, 1.0)):
            for t in range(4):
                for s in range(2):
                    pp = ps_big.tile([128, 512], F32, tag="S")
                    for c in range(4):
                        nc.tensor.matmul(
                            pp[:],
                            lhsT=w_sb[:, c, t * 128:(t + 1) * 128],
                            rhs=x_sb[:, c, s * 512:(s + 1) * 512],
                            start=(c == 0), stop=(c == 3))
                    if dst is qTs:
                        nc.scalar.mul(dst[:, t, s * 512:(s + 1) * 512], pp[:], scale)
                    else:
                        nc.vector.tensor_copy(dst[:, t, s * 512:(s + 1) * 512], pp[:])
        for lt in range(8):
            pp = ps_big.tile([128, 512], F32, tag="S")
            for c in range(4):
                nc.tensor.matmul(
                    pp[:],
                    lhsT=xv[:, c, lt * 128:(lt + 1) * 128],
                    rhs=wv[:, c, :],
                    start=(c == 0), stop=(c == 3))
            if lt % 2 == 0:
                nc.vector.tensor_copy(v_sb[:, lt, :], pp[:])
            else:
                nc.scalar.copy(v_sb[:, lt, :], pp[:])

        # ---- attention
        for qt in range(8):
            Lq0 = qt * 128
            Lk = (qt + 1) * 128
            sums = sb.tile([128, 8], F32, tag="sums")
            out_sb = sb.tile([128, HD], F32, tag="osb")
            for h in range(8):
                t, po = h // 2, (h % 2) * 64
                S = ps_big.tile([128, 1024], F32, tag="S")
                for w in range(0, Lk, 512):
                    n = min(512, Lk - w)
                    diag = (w + n == Lk)
                    nc.tensor.matmul(
                        S[:, w:w + n],
                        lhsT=qTs[po:po + 64, t, Lq0:Lq0 + 128],
                        rhs=kTs[po:po + 64, t, w:w + n],
                        start=True, stop=not diag)
                    if diag:
                        nc.tensor.matmul(S[:, Lk - 128:Lk], lhsT=cmaskT[:],
                                         rhs=ident[:], start=False, stop=True)
                nm = sb.tile([128, 1], F32, tag="nm")
                nc.vector.reduce_max(nm[:], S[:, :Lk], axis=mybir.AxisListType.X,
                                     negate=True)
                pr = sb.tile([128, 1024], F16, tag="pr")
                nc.scalar.activation(pr[:, :Lk], S[:, :Lk],
                                     mybir.ActivationFunctionType.Exp,
                                     bias=nm[:], scale=1.0,
                                     accum_out=sums[:, h:h + 1])
                pT = ps_sm.tile([128, 1024], F16, tag="pT")
                for kc in range(qt + 1):
                    nc.tensor.transpose(pT[:, kc * 128:(kc + 1) * 128],
                                        pr[:, kc * 128:(kc + 1) * 128], ident[:])
                pTs = sb.tile([128, 1024], F16, tag="pTs")
                nc.vector.tensor_copy(pTs[:, :Lk], pT[:, :Lk])
                av = ps_sm.tile([128, 64], F32, tag="av")
                for kc in range(qt + 1):
                    nc.tensor.matmul(av[:], lhsT=pTs[:, kc * 128:(kc + 1) * 128],
                                     rhs=v_sb[:, kc, h * 64:(h + 1) * 64],
                                     start=(kc == 0), stop=(kc == qt))
                nc.scalar.copy(out_sb[:, h * 64:(h + 1) * 64], av[:])
            rec = sb.tile([128, 8], F32, tag="rec")
            nc.vector.reciprocal(rec[:], sums[:])
            for h in range(8):
                nc.vector.tensor_scalar_mul(out_sb[:, h * 64:(h + 1) * 64],
                                            out_sb[:, h * 64:(h + 1) * 64],
                                            rec[:, h:h + 1])
            nc.sync.dma_start(out_d[Lq0:Lq0 + 128, :], out_sb[:])


def kernel(Q_seq, K_seq, V_seq, WQ, WK, WV, _trace=False):
    if "nc" not in _cached:
        _cached["nc"] = _build()
    nc = _cached["nc"]

    wq16 = np.asarray(WQ, dtype=np.float16)
    wk16 = np.asarray(WK, dtype=np.float16)
    wv16 = np.asarray(WV, dtype=np.float16)
    in_maps = []
    for b in range(N_CORES):
        in_maps.append({
            "qt": np.ascontiguousarray(np.asarray(Q_seq[b]).T.astype(np.float16)),
            "kt": np.ascontiguousarray(np.asarray(K_seq[b]).T.astype(np.float16)),
            "vt": np.ascontiguousarray(np.asarray(V_seq[b]).T.astype(np.float16)),
            "wq": wq16, "wk": wk16, "wv": wv16,
        })
    res = run_bass_kernel_spmd(nc, in_maps, core_ids=list(range(N_CORES)),
                               trace=_trace)
    out = np.stack([res.results[b]["out"] for b in range(N_CORES)], axis=0)
    if _trace:
        kernel.last_exec_time_ns = res.exec_time_ns
        kernel.last_results = res
    return out
